# revision 21
# baseline (speedup 1.0000x reference)
"""Trainium2 Bass kernel for CliffordFrameAttention (v3).

Sharding: 8 cores = 2 batches x 4 head-pairs. Each core computes two full
attention heads (L=2048 queries x 2048 keys) for one batch element plus the
fused Clifford geometric products, and emits a per-core partial [32, 2048]
output^T (Wo folded into the Cayley tensor per head). The host transposes and
sums the four head-pair partials per batch.

Algebra (validated in numpy, rel err ~1e-6):
  P = exp(S) masked (via min with {inf,0} mask), rs = row sums (ones-column in
  the PV matmul), Vu = V'^T @ P.
  U'' = Vu * (1/rs) + 0.25*K   (per head)
  out^T = sum_h [ gp(Qs, U'')@Wo_h^T  (Cayley fold, sqrt(D) corrected)
                  + Wo_h^T-term via U'' ]  - 0.25*(x@W2sum)^T
  1/rs computed packed ([128,16] reciprocal), PE-transposed to [16,128],
  bounced through DRAM into a 32-partition broadcast strip.

Schedule: phase C for query-half qh runs concurrently with phase B of the
other half (list scheduler + per-half data deps). Elementwise work is split
DVE/GpSimd. Final output accumulates in a single PSUM strip [32, 1024] per
half including both heads and the W2 correction; one DVE copy + DMA out.
"""

import math
import sys

for _p in ("/opt/trn_rl_repo", "/opt/trn_rl_repo/concourse"):
    if _p not in sys.path:
        sys.path.insert(0, _p)

import numpy as np
import ml_dtypes

import concourse.bass as bass
import concourse.mybir as mybir
import concourse.tile as tile
from concourse import bacc
from concourse import library_config
from concourse.bass_utils import run_bass_kernel_spmd

BF16 = ml_dtypes.bfloat16
F32 = mybir.dt.float32
F32R = mybir.dt.float32r
BF = mybir.dt.bfloat16

N_CORES = 8
B, L, D = 2, 2048, 32
H = 8
NC16 = 16          # number of 128-row chunks of L
LH = L // 2        # query-half width (1024)

_compiled_nc = None
LAST_RESULT = None

# engine split knobs (gpsimd elementwise unsupported by this codegen path)
MASK_GP_EVERY = 0   # disabled: all mask ops on DVE
T_GP_SET = ()       # disabled: all T-multiplies on DVE


def _build():
    nc = bacc.Bacc("TRN2", target_bir_lowering=False, debug=False,
                   num_devices=N_CORES)

    # ---- I/O ----
    xT_d = nc.declare_dram_parameter("xT", [32, L], F32R, isOutput=False)
    maskT_d = nc.declare_dram_parameter("maskT", [L, L], BF, isOutput=False)
    wqk_d = nc.declare_dram_parameter("wqk", [32, 192], F32R, isOutput=False)
    wv_d = nc.declare_dram_parameter("wv", [32, 64], F32R, isOutput=False)
    w2neg_d = nc.declare_dram_parameter("w2neg", [32, 32], F32R, isOutput=False)
    cp_d = nc.declare_dram_parameter("cp", [1024, 64], BF, isOutput=False)
    woT2_d = nc.declare_dram_parameter("woT2", [128, 32], BF, isOutput=False)
    id128_d = nc.declare_dram_parameter("id128", [128, 128], F32, isOutput=False)
    out_d = nc.declare_dram_parameter("out", [32, L], F32, isOutput=True)

    qT_dram = nc.dram_tensor("qT_bounce", [2, 32, L], BF)
    inv_dram = nc.dram_tensor("inv_bounce", [2, 16, 128], BF)  # [qh]

    with tile.TileContext(nc) as tc:
        with (
            tc.tile_pool(name="const", bufs=1) as cpool,
            tc.tile_pool(name="pt", bufs=6) as ptpool,
            tc.tile_pool(name="qrep", bufs=12) as qrpool,
            tc.tile_pool(name="tbuf", bufs=6) as tpool,
            tc.tile_pool(name="small", bufs=2) as spool,
            tc.tile_pool(name="uu", bufs=2) as upool,
            tc.tile_pool(name="psv", bufs=1, space="PSUM") as psvpool,
            tc.tile_pool(name="pssc", bufs=2, space="PSUM") as pssc,
            tc.tile_pool(name="psgp", bufs=1, space="PSUM") as psgppool,
        ):
            # ---------- constants / inputs into SBUF ----------
            xT = cpool.tile([32, L], F32R, tag="xT")
            nc.sync.dma_start(out=xT[:], in_=xT_d[:])
            wqk = cpool.tile([32, 192], F32R, tag="wqk")
            nc.sync.dma_start(out=wqk[:], in_=wqk_d[:])
            wv = cpool.tile([32, 64], F32R, tag="wv")
            nc.sync.dma_start(out=wv[:], in_=wv_d[:])
            w2neg = cpool.tile([32, 32], F32R, tag="w2neg")
            nc.sync.dma_start(out=w2neg[:], in_=w2neg_d[:])
            cp_sb = cpool.tile([128, 8, 64], BF, tag="cp")
            for a in range(8):
                nc.sync.dma_start(out=cp_sb[:, a, :], in_=cp_d[128 * a:128 * a + 128, :])
            woT2 = cpool.tile([128, 32], BF, tag="woT2")
            nc.sync.dma_start(out=woT2[:], in_=woT2_d[:])
            id128 = cpool.tile([128, 128], F32, tag="id128")
            nc.sync.dma_start(out=id128[:], in_=id128_d[:])

            # full mask resident in SBUF ({inf, 0} as bf16), loaded once
            mask_sb = cpool.tile([128, NC16, L], BF, tag="mask")
            _dma_cycle = (nc.sync, nc.scalar, nc.gpsimd)
            for c in range(NC16):
                eng = _dma_cycle[c % 3]
                eng.dma_start(out=mask_sb[:, c, :],
                              in_=maskT_d[128 * c:128 * c + 128, :])

            # persistent SBUF state
            qku = cpool.tile([32, 4 * L], BF, tag="qku")     # [Q_h0|Kg_h0|Q_h1|Kg_h1]
            q2b = cpool.tile([64, 2, L], BF, tag="q2b")      # rows 32-63 used
            kg2o = cpool.tile([64, 2, L // 2], BF, tag="kg2o")  # rows 32-63: odd chunks
            proj_l = cpool.tile([128, NC16, 66], BF, tag="projl")
            k25 = cpool.tile([128, L], BF, tag="k25")        # rows 0-31 h0, 64-95 h1
            urep0 = cpool.tile([128, L], BF, tag="urep0")
            urep1 = cpool.tile([128, L], BF, tag="urep1")
            urep = [urep0, urep1]
            invs = cpool.tile([128, LH], BF, tag="invs")     # rows 0-31 h0, 64-95 h1
            invT = cpool.tile([16, 128], BF, tag="invT")
            vu_sb = cpool.tile([128, LH], BF, tag="vusb")    # rows 0-31 h0, 64-95 h1
            rs_st = cpool.tile([1, 2, LH], F32, tag="rsst")
            ones1 = cpool.tile([1, 16], F32, tag="ones1")
            out_sb = cpool.tile([32, L], F32, tag="outsb")
            nc.gpsimd.memset(ones1[:], 1.0)

            # ones columns of V' (cols 32 and 65 of each chunk window)
            nc.gpsimd.memset(proj_l[:, :, 32:33], 1.0)
            nc.gpsimd.memset(proj_l[:, :, 65:66], 1.0)

            # ---------- phase A: projections ----------
            # Q^T / Kg^T for both heads -> qku [32, 4L]
            for h in range(2):
                for t in range(2):  # 0 = Q, 1 = Kg
                    for lh in range(2):
                        ps_qk_t = pssc.tile([128, 1024], F32, tag="sc")
                        ps_qk = ps_qk_t[0:32, :]
                        for nt in range(2):
                            nc.tensor.matmul(
                                ps_qk[:, 512 * nt:512 * nt + 512],
                                wqk[:, 64 * h + 32 * t:64 * h + 32 * t + 32],
                                xT[:, 1024 * lh + 512 * nt:1024 * lh + 512 * nt + 512],
                                start=True, stop=True,
                            )
                        dst = qku[:, L * (2 * h + t) + 1024 * lh:L * (2 * h + t) + 1024 * lh + 1024]
                        nc.vector.tensor_copy(out=dst, in_=ps_qk[:])
            # Q^T (bf16) to DRAM bounce for later broadcast-replication
            for h in range(2):
                nc.sync.dma_start(out=qT_dram[h], in_=qku[:, L * 2 * h:L * 2 * h + L])

            # strip-32 copies for PE row-tiling (odd-chunk score tile at (32,0))
            for h in range(2):
                qw = qku[:, L * 2 * h:L * 2 * h + L]
                kw = qku[:, L * (2 * h + 1):L * (2 * h + 1) + L]
                nc.vector.tensor_copy(out=q2b[32:64, h, :], in_=qw)
                kger = kw.rearrange("p (g two c) -> p g two c", two=2, c=128)
                k2r = kg2o[:, h, :].rearrange("p (g c) -> p g c", c=128)
                nc.vector.tensor_copy(out=k2r[32:64], in_=kger[:, :, 1, :])

            # qrep broadcast prefetch: tiles fill as slots free, DMAs spread
            # over the three DMA-capable engines (deps: qT_dram only)
            qr_tiles = {}
            _qi = 0
            for qh in range(2):
                for h in range(2):
                    for a in range(8):
                        qr = qrpool.tile([128, LH], BF, tag="qr")
                        for i in range(4):
                            eng = _dma_cycle[_qi % 3]
                            _qi += 1
                            eng.dma_start(
                                out=qr[32 * i:32 * i + 32, :],
                                in_=qT_dram[h][4 * a + i:4 * a + i + 1,
                                               LH * qh:LH * qh + LH]
                                .to_broadcast([32, LH]),
                            )
                        qr_tiles[(qh, h, a)] = qr

            # ---------- main: both halves ----------
            mask_ctr = 0
            for qh in range(2):
                qwin = slice(LH * qh, LH * qh + LH)
                # ---- phase B: attention for this half ----
                ps_vu = psvpool.tile([128, LH], F32, tag="vu")
                for cp_i in range(NC16 // 2):
                    ce, co = 2 * cp_i, 2 * cp_i + 1
                    if qh == 0:
                        # V projections just-in-time (off the startup path)
                        for c in (ce, co):
                            ps_v_t = pssc.tile([128, LH], F32, tag="sc")
                            ps_v = ps_v_t[:, 0:64]
                            nc.tensor.matmul(
                                ps_v[:],
                                xT[:, 128 * c:128 * c + 128],
                                wv[:],
                                start=True, stop=True,
                            )
                            nc.vector.tensor_copy(
                                out=proj_l[:, c, 0:66]
                                .rearrange("p (a b) -> p a b", a=2)[:, :, 0:32],
                                in_=ps_v[:].rearrange("p (a b) -> p a b", a=2),
                            )
                    pt_e = ptpool.tile([128, 2, LH], BF, tag="pt")
                    pt_o = ptpool.tile([128, 2, LH], BF, tag="pt")
                    for h in range(2):
                        sc_e = pssc.tile([128, LH], F32, tag="sc")
                        sc_o = pssc.tile([128, LH], F32, tag="sc")
                        for nt in range(2):
                            qs = qku[:, L * 2 * h + LH * qh + 512 * nt:
                                     L * 2 * h + LH * qh + 512 * nt + 512]
                            nc.tensor.matmul(
                                sc_e[:, 512 * nt:512 * nt + 512],
                                qku[:, L * (2 * h + 1) + 256 * cp_i:
                                    L * (2 * h + 1) + 256 * cp_i + 128],
                                qs, start=True, stop=True,
                            )
                        for nt in range(2):
                            qs = q2b[32:64, h, LH * qh + 512 * nt:LH * qh + 512 * nt + 512]
                            nc.tensor.matmul(
                                sc_o[:, 512 * nt:512 * nt + 512],
                                kg2o[32:64, h, 128 * cp_i:128 * cp_i + 128],
                                qs, start=True, stop=True,
                            )
                        nc.scalar.activation(pt_e[:, h, :], sc_e[:],
                                             mybir.ActivationFunctionType.Exp)
                        nc.scalar.activation(pt_o[:, h, :], sc_o[:],
                                             mybir.ActivationFunctionType.Exp)
                    # mask via min against {inf, 0}
                    for pt_t, c in ((pt_e, ce), (pt_o, co)):
                        m_ap = mask_sb[:, c, qwin]
                        m_j = m_ap.unsqueeze(1).broadcast_to([128, 2, LH])
                        nc.vector.tensor_tensor(
                            out=pt_t[:], in0=pt_t[:], in1=m_j,
                            op=mybir.AluOpType.min)
                        mask_ctr += 1
                    # PV: col tiles (0,0) for h0, (0,64) for h1, per chunk
                    for cidx, pt_t in ((ce, pt_e), (co, pt_o)):
                        for h in range(2):
                            v0 = 64 * h
                            for nt in range(2):
                                nc.tensor.matmul(
                                    ps_vu[v0:v0 + 33, 512 * nt:512 * nt + 512],
                                    proj_l[:, cidx, 33 * h:33 * h + 33],
                                    pt_t[:, h, 512 * nt:512 * nt + 512],
                                    start=(cp_i == 0 and cidx == ce),
                                    stop=(cp_i == NC16 // 2 - 1 and cidx == co),
                                )

                if qh == 0:
                    # K25 = 0.25*K^T per head -> k25 (needed from the epilogue
                    # on; emitted late to keep the startup path short)
                    for h in range(2):
                        for lh in range(2):
                            ps_k_t = pssc.tile([128, LH], F32, tag="sc")
                            ps_k = ps_k_t[0:32, :]
                            for nt in range(2):
                                nc.tensor.matmul(
                                    ps_k[:, 512 * nt:512 * nt + 512],
                                    wqk[:, 128 + 32 * h:128 + 32 * h + 32],
                                    xT[:, 1024 * lh + 512 * nt:
                                       1024 * lh + 512 * nt + 512],
                                    start=True, stop=True,
                                )
                            nc.vector.tensor_copy(
                                out=k25[64 * h:64 * h + 32,
                                        1024 * lh:1024 * lh + 1024],
                                in_=ps_k[:])

                # ---- epilogue: rs -> invrs strip, U'' assembly ----
                # stage rs rows + Vu strips to SBUF fast: frees ps_vu for the
                # next half's PV accumulation.
                for h in range(2):
                    v0 = 64 * h
                    nc.vector.tensor_copy(out=rs_st[0:1, h, :],
                                          in_=ps_vu[v0 + 32:v0 + 33, :])
                for h in range(2):
                    v0 = 64 * h
                    nc.vector.tensor_copy(out=vu_sb[v0:v0 + 32, :],
                                          in_=ps_vu[v0:v0 + 32, :])
                # epilogue PSUM scratch lives inside the gp tile (keeps the
                # score pool free for the next half); AP overlaps serialize
                # the bank-clearing W2 matmuls behind the scratch readers.
                ps_gp_t = psgppool.tile([128, LH], F32, tag="gp")
                # pack rs into [128, 16] via tiny row-transpose matmuls
                ps_rs = ps_gp_t[:, 0:16]
                for h in range(2):
                    for cl in range(8):
                        nc.tensor.matmul(
                            ps_rs[:, 8 * h + cl:8 * h + cl + 1],
                            rs_st[0:1, h, 128 * cl:128 * cl + 128],
                            ones1[0:1, 0:1],
                            start=(h == 0 and cl == 0),
                            stop=(h == 1 and cl == 7),
                            skip_group_check=True,
                        )
                invp = spool.tile([128, 16], F32, tag="invp")
                nc.vector.tensor_scalar(invp[:], ps_rs, 1e-30, None,
                                        op0=mybir.AluOpType.add)
                nc.vector.reciprocal(invp[:], invp[:])
                ps_tr = ps_gp_t[0:16, 0:128]
                nc.tensor.transpose(ps_tr, invp[:], id128[:],
                                    tile_position=(0, 0))
                nc.vector.tensor_copy(out=invT[:], in_=ps_tr)
                nc.sync.dma_start(out=inv_dram[qh], in_=invT[:])
                for h in range(2):
                    v0 = 64 * h
                    src = inv_dram[qh][8 * h:8 * h + 8].rearrange("a b -> (a b)") \
                        .unsqueeze(0)
                    nc.sync.dma_start(out=invs[v0:v0 + 32, :],
                                      in_=src.to_broadcast([32, LH]))
                    uu = upool.tile([128, LH], BF, tag="uu")
                    nc.vector.tensor_tensor(
                        out=uu[v0:v0 + 32, :], in0=vu_sb[v0:v0 + 32, :],
                        in1=invs[v0:v0 + 32, :], op=mybir.AluOpType.mult)
                    nc.vector.tensor_tensor(
                        out=urep[h][v0:v0 + 32, qwin], in0=uu[v0:v0 + 32, :],
                        in1=k25[v0:v0 + 32, qwin], op=mybir.AluOpType.add)
                    ei = 0
                    for r in range(4):
                        if 32 * r == v0:
                            continue
                        _dma_cycle[ei].dma_start(
                            out=urep[h][32 * r:32 * r + 32, qwin],
                            in_=urep[h][v0:v0 + 32, qwin])
                        ei += 1

                # ---- phase C: geometric products for this half ----
                ps_gp = ps_gp_t[0:32, :]
                for nt in range(2):
                    nc.tensor.matmul(
                        ps_gp[:, 512 * nt:512 * nt + 512],
                        w2neg[:],
                        xT[:, LH * qh + 512 * nt:LH * qh + 512 * nt + 512],
                        start=True, stop=False, skip_group_check=True,
                    )
                for h in range(2):
                    v0 = 64 * h
                    for a in range(8):
                        qr = qr_tiles[(qh, h, a)]
                        t_a = tpool.tile([128, LH], BF, tag="tt")
                        nc.vector.tensor_tensor(out=t_a[:], in0=qr[:],
                                                in1=urep[h][:, qwin],
                                                op=mybir.AluOpType.mult)
                        for nt in range(2):
                            nc.tensor.matmul(
                                ps_gp[:, 512 * nt:512 * nt + 512],
                                cp_sb[:, a, 32 * h:32 * h + 32],
                                t_a[:, 512 * nt:512 * nt + 512],
                                start=False, stop=False, skip_group_check=True,
                            )
                    for nt in range(2):
                        nc.tensor.matmul(
                            ps_gp[:, 512 * nt:512 * nt + 512],
                            woT2[v0:v0 + 32, :],
                            urep[h][v0:v0 + 32,
                                    LH * qh + 512 * nt:LH * qh + 512 * nt + 512],
                            start=False, stop=(h == 1 and nt == 1),
                            skip_group_check=True,
                        )
                nc.vector.tensor_copy(out=out_sb[:, qwin], in_=ps_gp[:])
                nc.sync.dma_start(out=out_d[:, qwin], in_=out_sb[:, qwin])

    nc.compile()
    return nc


def _get_nc():
    global _compiled_nc
    if _compiled_nc is None:
        _compiled_nc = _build()
    return _compiled_nc


def kernel(x, mask, Wq, Wk, Wv, Wo, cayley, grade_signs):
    x = np.asarray(x, dtype=np.float32)
    mask = np.asarray(mask)
    Wq = np.asarray(Wq, dtype=np.float32)
    Wk = np.asarray(Wk, dtype=np.float32)
    Wv = np.asarray(Wv, dtype=np.float32)
    Wo = np.asarray(Wo, dtype=np.float32)
    cayley = np.asarray(cayley, dtype=np.float32)
    gs = np.asarray(grade_signs, dtype=np.float32)

    s = 1.0 / math.sqrt(D)
    id128 = np.eye(128, dtype=np.float32)

    in_maps = []
    for core in range(N_CORES):
        b, hp = core // 4, core % 4
        heads = (2 * hp, 2 * hp + 1)
        xT = np.ascontiguousarray(x[b].T)
        maskinf = np.where(np.ascontiguousarray(mask[b].T),
                           np.float32(np.inf), np.float32(0.0)).astype(BF16)

        wqk = np.zeros((32, 192), np.float32)
        wv_p = np.zeros((32, 64), np.float32)
        cp = np.zeros((1024, 64), np.float32)
        woT2 = np.zeros((128, 32), np.float32)
        W2sum = np.zeros((32, 32), np.float32)
        for j, h in enumerate(heads):
            Wq_h = Wq[32 * h:32 * h + 32]
            Wk_h = Wk[32 * h:32 * h + 32]
            Wv_h = Wv[32 * h:32 * h + 32]
            Wo_h = Wo[:, 32 * h:32 * h + 32]
            wqk[:, 64 * j:64 * j + 32] = Wq_h.T * s
            wqk[:, 64 * j + 32:64 * j + 64] = Wk_h.T * gs[None, :]
            wqk[:, 128 + 32 * j:128 + 32 * j + 32] = 0.25 * Wk_h.T
            wv_p[:, 32 * j:32 * j + 32] = Wv_h.T
            W2sum += Wk_h.T @ Wo_h.T
            cp[:, 32 * j:32 * j + 32] = (
                math.sqrt(D) * np.einsum('ijk,dk->ijd', cayley, Wo_h)
            ).reshape(1024, 32)
            woT2[64 * j:64 * j + 32, :] = Wo_h.T
        w2neg = (-0.25 * W2sum).astype(np.float32)

        in_maps.append({
            "xT": xT,
            "maskT": maskinf,
            "wqk": wqk,
            "wv": wv_p,
            "w2neg": w2neg,
            "cp": cp.astype(BF16),
            "woT2": woT2.astype(BF16),
            "id128": id128,
        })

    import os as _os
    _trace = bool(_os.environ.get("KTRACE"))
    res = run_bass_kernel_spmd(_get_nc(), in_maps, list(range(N_CORES)),
                               trace=_trace)
    global LAST_RESULT
    LAST_RESULT = res
    out = np.zeros((B, L, D), np.float32)
    for core in range(N_CORES):
        out[core // 4] += res.results[core]["out"].T
    return out


# revision 29
# speedup vs baseline: 1.2062x; 1.2062x over previous
"""Trainium2 Bass kernel for CliffordFrameAttention (v3).

Sharding: 8 cores = 2 batches x 4 head-pairs. Each core computes two full
attention heads (L=2048 queries x 2048 keys) for one batch element plus the
fused Clifford geometric products, and emits a per-core partial [32, 2048]
output^T (Wo folded into the Cayley tensor per head). The host transposes and
sums the four head-pair partials per batch.

Algebra (validated in numpy, rel err ~1e-6):
  P = exp(S) masked (via min with {inf,0} mask), rs = row sums (ones-column in
  the PV matmul), Vu = V'^T @ P.
  U'' = Vu * (1/rs) + 0.25*K   (per head)
  out^T = sum_h [ gp(Qs, U'')@Wo_h^T  (Cayley fold, sqrt(D) corrected)
                  + Wo_h^T-term via U'' ]  - 0.25*(x@W2sum)^T
  1/rs computed packed ([128,16] reciprocal), PE-transposed to [16,128],
  bounced through DRAM into a 32-partition broadcast strip.

Schedule: phase C for query-half qh runs concurrently with phase B of the
other half (list scheduler + per-half data deps). Elementwise work is split
DVE/GpSimd. Final output accumulates in a single PSUM strip [32, 1024] per
half including both heads and the W2 correction; one DVE copy + DMA out.
"""

import math
import sys

for _p in ("/opt/trn_rl_repo", "/opt/trn_rl_repo/concourse"):
    if _p not in sys.path:
        sys.path.insert(0, _p)

import numpy as np
import ml_dtypes

import concourse.bass as bass
import concourse.mybir as mybir
import concourse.tile as tile
from concourse import bacc
from concourse import library_config
from concourse.bass_utils import run_bass_kernel_spmd

BF16 = ml_dtypes.bfloat16
F32 = mybir.dt.float32
F32R = mybir.dt.float32r
BF = mybir.dt.bfloat16

N_CORES = 8
B, L, D = 2, 2048, 32
H = 8
NC16 = 16          # number of 128-row chunks of L
LH = L // 2        # query-half width (1024)

_compiled_nc = None
LAST_RESULT = None

# engine split knobs (gpsimd elementwise unsupported by this codegen path)
MASK_GP_EVERY = 0   # disabled: all mask ops on DVE
T_GP_SET = ()       # disabled: all T-multiplies on DVE


def _build():
    nc = bacc.Bacc("TRN2", target_bir_lowering=False, debug=False,
                   num_devices=N_CORES)

    # ---- I/O ----
    xT_d = nc.declare_dram_parameter("xT", [32, L], F32R, isOutput=False)
    maskT_d = nc.declare_dram_parameter("maskT", [L, L], BF, isOutput=False)
    wqk_d = nc.declare_dram_parameter("wqk", [32, 192], F32R, isOutput=False)
    wv_d = nc.declare_dram_parameter("wv", [32, 64], F32R, isOutput=False)
    w2neg_d = nc.declare_dram_parameter("w2neg", [32, 32], F32R, isOutput=False)
    cp_d = nc.declare_dram_parameter("cp", [1024, 64], BF, isOutput=False)
    woT2_d = nc.declare_dram_parameter("woT2", [128, 32], BF, isOutput=False)
    id128_d = nc.declare_dram_parameter("id128", [128, 128], F32, isOutput=False)
    out_d = nc.declare_dram_parameter("out", [32, L], F32, isOutput=True)

    qT_dram = nc.dram_tensor("qT_bounce", [2, 32, L], BF)
    inv_dram = nc.dram_tensor("inv_bounce", [2, 16, 128], BF)  # [qh]

    with tile.TileContext(nc) as tc:
        with (
            tc.tile_pool(name="const", bufs=1) as cpool,
            tc.tile_pool(name="pt", bufs=6) as ptpool,
            tc.tile_pool(name="qrep", bufs=12) as qrpool,
            tc.tile_pool(name="tbuf", bufs=6) as tpool,
            tc.tile_pool(name="small", bufs=2) as spool,
            tc.tile_pool(name="uu", bufs=2) as upool,
            tc.tile_pool(name="psv", bufs=1, space="PSUM") as psvpool,
            tc.tile_pool(name="pssc", bufs=2, space="PSUM") as pssc,
            tc.tile_pool(name="psgp", bufs=1, space="PSUM") as psgppool,
        ):
            # ---------- constants / inputs into SBUF ----------
            xT = cpool.tile([32, L], F32R, tag="xT")
            nc.sync.dma_start(out=xT[:], in_=xT_d[:])
            wqk = cpool.tile([32, 192], F32R, tag="wqk")
            nc.sync.dma_start(out=wqk[:], in_=wqk_d[:])
            wv = cpool.tile([32, 64], F32R, tag="wv")
            nc.sync.dma_start(out=wv[:], in_=wv_d[:])
            w2neg = cpool.tile([32, 32], F32R, tag="w2neg")
            nc.sync.dma_start(out=w2neg[:], in_=w2neg_d[:])
            cp_sb = cpool.tile([128, 8, 64], BF, tag="cp")
            for a in range(8):
                nc.sync.dma_start(out=cp_sb[:, a, :], in_=cp_d[128 * a:128 * a + 128, :])
            woT2 = cpool.tile([128, 32], BF, tag="woT2")
            nc.sync.dma_start(out=woT2[:], in_=woT2_d[:])
            id128 = cpool.tile([128, 128], F32, tag="id128")
            nc.sync.dma_start(out=id128[:], in_=id128_d[:])

            # full mask resident in SBUF ({inf, 0} as bf16), loaded once
            mask_sb = cpool.tile([128, NC16, L], BF, tag="mask")
            for c in range(NC16):
                eng = (nc.sync, nc.scalar)[c % 2]
                eng.dma_start(out=mask_sb[:, c, :],
                              in_=maskT_d[128 * c:128 * c + 128, :])

            # persistent SBUF state
            qku = cpool.tile([32, 4 * L], BF, tag="qku")     # [Q_h0|Kg_h0|Q_h1|Kg_h1]
            q2b = cpool.tile([64, 2, L], BF, tag="q2b")      # rows 32-63 used
            kg2o = cpool.tile([64, 2, L // 2], BF, tag="kg2o")  # rows 32-63: odd chunks
            proj_l = cpool.tile([128, NC16, 66], BF, tag="projl")
            k25 = cpool.tile([128, L], BF, tag="k25")        # rows 0-31 h0, 64-95 h1
            urep0 = cpool.tile([128, L], BF, tag="urep0")
            urep1 = cpool.tile([128, L], BF, tag="urep1")
            urep = [urep0, urep1]
            invs = cpool.tile([128, LH], BF, tag="invs")     # rows 0-31 h0, 64-95 h1
            invT = cpool.tile([16, 128], BF, tag="invT")
            vu_sb = cpool.tile([128, LH], BF, tag="vusb")    # rows 0-31 h0, 64-95 h1
            rs_st = cpool.tile([1, 2, LH], F32, tag="rsst")
            ones1 = cpool.tile([1, 16], F32, tag="ones1")
            out_sb = cpool.tile([32, L], F32, tag="outsb")
            nc.gpsimd.memset(ones1[:], 1.0)

            # ones columns of V' (cols 32 and 65 of each chunk window)
            nc.gpsimd.memset(proj_l[:, :, 32:33], 1.0)
            nc.gpsimd.memset(proj_l[:, :, 65:66], 1.0)

            # ---------- phase A: projections ----------
            # Q^T / Kg^T for both heads -> qku [32, 4L]
            for h in range(2):
                for t in range(2):  # 0 = Q, 1 = Kg
                    for lh in range(2):
                        ps_qk_t = pssc.tile([128, 1024], F32, tag="sc")
                        ps_qk = ps_qk_t[0:32, :]
                        for nt in range(2):
                            nc.tensor.matmul(
                                ps_qk[:, 512 * nt:512 * nt + 512],
                                wqk[:, 64 * h + 32 * t:64 * h + 32 * t + 32],
                                xT[:, 1024 * lh + 512 * nt:1024 * lh + 512 * nt + 512],
                                start=True, stop=True,
                            )
                        dst = qku[:, L * (2 * h + t) + 1024 * lh:L * (2 * h + t) + 1024 * lh + 1024]
                        nc.vector.tensor_copy(out=dst, in_=ps_qk[:])
            # Q^T (bf16) to DRAM bounce for later broadcast-replication
            # (gpsimd SWDGE: reserved for latency-critical small DMAs)
            for h in range(2):
                nc.gpsimd.dma_start(out=qT_dram[h],
                                    in_=qku[:, L * 2 * h:L * 2 * h + L])

            # strip-32 copies for PE row-tiling (odd-chunk score tile at (32,0))
            for h in range(2):
                qw = qku[:, L * 2 * h:L * 2 * h + L]
                kw = qku[:, L * (2 * h + 1):L * (2 * h + 1) + L]
                nc.vector.tensor_copy(out=q2b[32:64, h, :], in_=qw)
                kger = kw.rearrange("p (g two c) -> p g two c", two=2, c=128)
                k2r = kg2o[:, h, :].rearrange("p (g c) -> p g c", c=128)
                nc.vector.tensor_copy(out=k2r[32:64], in_=kger[:, :, 1, :])

            def emit_qrep(qh):
                """qrep broadcasts for one half; bulk rings (sync/scalar)."""
                tiles = {}
                qi = 0
                for h in range(2):
                    for a in range(8):
                        qr = qrpool.tile([128, LH], BF, tag="qr")
                        for i in range(4):
                            eng = (nc.sync, nc.scalar)[qi % 2]
                            qi += 1
                            eng.dma_start(
                                out=qr[32 * i:32 * i + 32, :],
                                in_=qT_dram[h][4 * a + i:4 * a + i + 1,
                                               LH * qh:LH * qh + LH]
                                .to_broadcast([32, LH]),
                            )
                        tiles[(h, a)] = qr
                return tiles

            # ---------- main: both halves ----------
            mask_ctr = 0
            for qh in range(2):
                qwin = slice(LH * qh, LH * qh + LH)
                # ---- phase B: attention for this half ----
                ps_vu = psvpool.tile([128, LH], F32, tag="vu")
                if qh == 0:
                    # V projections packed into the (idle) gp-pool scratch so
                    # they never contend for the score PSUM slots
                    ps_vsc_t = psgppool.tile([128, LH], F32, tag="gp")
                for cp_i in range(NC16 // 2):
                    ce, co = 2 * cp_i, 2 * cp_i + 1
                    if qh == 0:
                        for c in (ce, co):
                            ps_v = ps_vsc_t[:, 64 * c:64 * c + 64]
                            nc.tensor.matmul(
                                ps_v,
                                xT[:, 128 * c:128 * c + 128],
                                wv[:],
                                start=(c % 8 == 0), stop=(c % 8 == 7),
                                skip_group_check=True,
                            )
                            nc.vector.tensor_copy(
                                out=proj_l[:, c, 0:66]
                                .rearrange("p (a b) -> p a b", a=2)[:, :, 0:32],
                                in_=ps_v.rearrange("p (a b) -> p a b", a=2),
                            )
                        if cp_i >= 4:
                            # K25 = 0.25*K^T, spread over late iterations
                            h25, lh25 = (cp_i - 4) // 2, (cp_i - 4) % 2
                            ps_k_t = pssc.tile([128, LH], F32, tag="sc")
                            ps_k = ps_k_t[0:32, :]
                            for nt in range(2):
                                nc.tensor.matmul(
                                    ps_k[:, 512 * nt:512 * nt + 512],
                                    wqk[:, 128 + 32 * h25:128 + 32 * h25 + 32],
                                    xT[:, 1024 * lh25 + 512 * nt:
                                       1024 * lh25 + 512 * nt + 512],
                                    start=True, stop=True,
                                )
                            nc.vector.tensor_copy(
                                out=k25[64 * h25:64 * h25 + 32,
                                        1024 * lh25:1024 * lh25 + 1024],
                                in_=ps_k[:])
                    pt_e = ptpool.tile([128, 2, LH], BF, tag="pt")
                    pt_o = ptpool.tile([128, 2, LH], BF, tag="pt")
                    for h in range(2):
                        sc_e = pssc.tile([128, LH], F32, tag="sc")
                        sc_o = pssc.tile([128, LH], F32, tag="sc")
                        for nt in range(2):
                            qs = qku[:, L * 2 * h + LH * qh + 512 * nt:
                                     L * 2 * h + LH * qh + 512 * nt + 512]
                            nc.tensor.matmul(
                                sc_e[:, 512 * nt:512 * nt + 512],
                                qku[:, L * (2 * h + 1) + 256 * cp_i:
                                    L * (2 * h + 1) + 256 * cp_i + 128],
                                qs, start=True, stop=True,
                            )
                        for nt in range(2):
                            qs = q2b[32:64, h, LH * qh + 512 * nt:LH * qh + 512 * nt + 512]
                            nc.tensor.matmul(
                                sc_o[:, 512 * nt:512 * nt + 512],
                                kg2o[32:64, h, 128 * cp_i:128 * cp_i + 128],
                                qs, start=True, stop=True,
                            )
                        nc.scalar.activation(pt_e[:, h, :], sc_e[:],
                                             mybir.ActivationFunctionType.Exp)
                        nc.scalar.activation(pt_o[:, h, :], sc_o[:],
                                             mybir.ActivationFunctionType.Exp)
                    # mask via min against {inf, 0}
                    for pt_t, c in ((pt_e, ce), (pt_o, co)):
                        m_ap = mask_sb[:, c, qwin]
                        m_j = m_ap.unsqueeze(1).broadcast_to([128, 2, LH])
                        nc.vector.tensor_tensor(
                            out=pt_t[:], in0=pt_t[:], in1=m_j,
                            op=mybir.AluOpType.min)
                        mask_ctr += 1
                    # PV: col tiles (0,0) for h0, (0,64) for h1, per chunk
                    for cidx, pt_t in ((ce, pt_e), (co, pt_o)):
                        for h in range(2):
                            v0 = 64 * h
                            for nt in range(2):
                                nc.tensor.matmul(
                                    ps_vu[v0:v0 + 33, 512 * nt:512 * nt + 512],
                                    proj_l[:, cidx, 33 * h:33 * h + 33],
                                    pt_t[:, h, 512 * nt:512 * nt + 512],
                                    start=(cp_i == 0 and cidx == ce),
                                    stop=(cp_i == NC16 // 2 - 1 and cidx == co),
                                )

                # qrep broadcasts for this half (bulk rings, after the half's
                # phase-B DMAs so they never starve mask/input loads)
                qr_tiles = emit_qrep(qh)

                # ---- epilogue: rs -> invrs strip, U'' assembly ----
                # stage rs rows + Vu strips to SBUF fast: frees ps_vu for the
                # next half's PV accumulation.
                for h in range(2):
                    v0 = 64 * h
                    nc.vector.tensor_copy(out=rs_st[0:1, h, :],
                                          in_=ps_vu[v0 + 32:v0 + 33, :])
                for h in range(2):
                    v0 = 64 * h
                    nc.vector.tensor_copy(out=vu_sb[v0:v0 + 32, :],
                                          in_=ps_vu[v0:v0 + 32, :])
                # epilogue PSUM scratch lives inside the gp tile (keeps the
                # score pool free for the next half); AP overlaps serialize
                # the bank-clearing W2 matmuls behind the scratch readers.
                ps_gp_t = psgppool.tile([128, LH], F32, tag="gp")
                # pack rs into [128, 16] via tiny row-transpose matmuls
                ps_rs = ps_gp_t[:, 0:16]
                for h in range(2):
                    for cl in range(8):
                        nc.tensor.matmul(
                            ps_rs[:, 8 * h + cl:8 * h + cl + 1],
                            rs_st[0:1, h, 128 * cl:128 * cl + 128],
                            ones1[0:1, 0:1],
                            start=(h == 0 and cl == 0),
                            stop=(h == 1 and cl == 7),
                            skip_group_check=True,
                        )
                invp = spool.tile([128, 16], F32, tag="invp")
                nc.vector.tensor_scalar(invp[:], ps_rs, 1e-30, None,
                                        op0=mybir.AluOpType.add)
                nc.vector.reciprocal(invp[:], invp[:])
                ps_tr = ps_gp_t[0:16, 0:128]
                nc.tensor.transpose(ps_tr, invp[:], id128[:],
                                    tile_position=(0, 0))
                nc.vector.tensor_copy(out=invT[:], in_=ps_tr)
                nc.gpsimd.dma_start(out=inv_dram[qh], in_=invT[:])
                for h in range(2):
                    v0 = 64 * h
                    src = inv_dram[qh][8 * h:8 * h + 8].rearrange("a b -> (a b)") \
                        .unsqueeze(0)
                    nc.gpsimd.dma_start(out=invs[v0:v0 + 32, :],
                                        in_=src.to_broadcast([32, LH]))
                    uu = upool.tile([128, LH], BF, tag="uu")
                    nc.vector.tensor_tensor(
                        out=uu[v0:v0 + 32, :], in0=vu_sb[v0:v0 + 32, :],
                        in1=invs[v0:v0 + 32, :], op=mybir.AluOpType.mult)
                    nc.vector.tensor_tensor(
                        out=urep[h][v0:v0 + 32, qwin], in0=uu[v0:v0 + 32, :],
                        in1=k25[v0:v0 + 32, qwin], op=mybir.AluOpType.add)
                    for r in range(4):
                        if 32 * r == v0:
                            continue
                        nc.gpsimd.dma_start(
                            out=urep[h][32 * r:32 * r + 32, qwin],
                            in_=urep[h][v0:v0 + 32, qwin])

                # ---- phase C: geometric products for this half ----
                ps_gp = ps_gp_t[0:32, :]
                for nt in range(2):
                    nc.tensor.matmul(
                        ps_gp[:, 512 * nt:512 * nt + 512],
                        w2neg[:],
                        xT[:, LH * qh + 512 * nt:LH * qh + 512 * nt + 512],
                        start=True, stop=False, skip_group_check=True,
                    )
                for h in range(2):
                    v0 = 64 * h
                    for a in range(8):
                        qr = qr_tiles[(h, a)]
                        t_a = tpool.tile([128, LH], BF, tag="tt")
                        nc.vector.tensor_tensor(out=t_a[:], in0=qr[:],
                                                in1=urep[h][:, qwin],
                                                op=mybir.AluOpType.mult)
                        for nt in range(2):
                            nc.tensor.matmul(
                                ps_gp[:, 512 * nt:512 * nt + 512],
                                cp_sb[:, a, 32 * h:32 * h + 32],
                                t_a[:, 512 * nt:512 * nt + 512],
                                start=False, stop=False, skip_group_check=True,
                            )
                    for nt in range(2):
                        nc.tensor.matmul(
                            ps_gp[:, 512 * nt:512 * nt + 512],
                            woT2[v0:v0 + 32, :],
                            urep[h][v0:v0 + 32,
                                    LH * qh + 512 * nt:LH * qh + 512 * nt + 512],
                            start=False, stop=(h == 1 and nt == 1),
                            skip_group_check=True,
                        )
                nc.vector.tensor_copy(out=out_sb[:, qwin], in_=ps_gp[:])
                nc.sync.dma_start(out=out_d[:, qwin], in_=out_sb[:, qwin])

    nc.compile()
    return nc


def _get_nc():
    global _compiled_nc
    if _compiled_nc is None:
        _compiled_nc = _build()
    return _compiled_nc


def kernel(x, mask, Wq, Wk, Wv, Wo, cayley, grade_signs):
    x = np.asarray(x, dtype=np.float32)
    mask = np.asarray(mask)
    Wq = np.asarray(Wq, dtype=np.float32)
    Wk = np.asarray(Wk, dtype=np.float32)
    Wv = np.asarray(Wv, dtype=np.float32)
    Wo = np.asarray(Wo, dtype=np.float32)
    cayley = np.asarray(cayley, dtype=np.float32)
    gs = np.asarray(grade_signs, dtype=np.float32)

    s = 1.0 / math.sqrt(D)
    id128 = np.eye(128, dtype=np.float32)

    in_maps = []
    for core in range(N_CORES):
        b, hp = core // 4, core % 4
        heads = (2 * hp, 2 * hp + 1)
        xT = np.ascontiguousarray(x[b].T)
        maskinf = np.where(np.ascontiguousarray(mask[b].T),
                           np.float32(np.inf), np.float32(0.0)).astype(BF16)

        wqk = np.zeros((32, 192), np.float32)
        wv_p = np.zeros((32, 64), np.float32)
        cp = np.zeros((1024, 64), np.float32)
        woT2 = np.zeros((128, 32), np.float32)
        W2sum = np.zeros((32, 32), np.float32)
        for j, h in enumerate(heads):
            Wq_h = Wq[32 * h:32 * h + 32]
            Wk_h = Wk[32 * h:32 * h + 32]
            Wv_h = Wv[32 * h:32 * h + 32]
            Wo_h = Wo[:, 32 * h:32 * h + 32]
            wqk[:, 64 * j:64 * j + 32] = Wq_h.T * s
            wqk[:, 64 * j + 32:64 * j + 64] = Wk_h.T * gs[None, :]
            wqk[:, 128 + 32 * j:128 + 32 * j + 32] = 0.25 * Wk_h.T
            wv_p[:, 32 * j:32 * j + 32] = Wv_h.T
            W2sum += Wk_h.T @ Wo_h.T
            cp[:, 32 * j:32 * j + 32] = (
                math.sqrt(D) * np.einsum('ijk,dk->ijd', cayley, Wo_h)
            ).reshape(1024, 32)
            woT2[64 * j:64 * j + 32, :] = Wo_h.T
        w2neg = (-0.25 * W2sum).astype(np.float32)

        in_maps.append({
            "xT": xT,
            "maskT": maskinf,
            "wqk": wqk,
            "wv": wv_p,
            "w2neg": w2neg,
            "cp": cp.astype(BF16),
            "woT2": woT2.astype(BF16),
            "id128": id128,
        })

    import os as _os
    _trace = bool(_os.environ.get("KTRACE"))
    res = run_bass_kernel_spmd(_get_nc(), in_maps, list(range(N_CORES)),
                               trace=_trace)
    global LAST_RESULT
    LAST_RESULT = res
    out = np.zeros((B, L, D), np.float32)
    for core in range(N_CORES):
        out[core // 4] += res.results[core]["out"].T
    return out
